# revision 44
# baseline (speedup 1.0000x reference)
"""GCN (3-layer) Bass kernel for 8 TRN2 NeuronCores, SPMD.

Math: out = A_hat @ relu(A_hat @ relu(A_hat @ X W1 + b1) W2 + b2) W3 + b3
A_hat = D^-1/2 (A + I) D^-1/2.

Key design (v2 — DVE-aggregated, lane-slotted gather):
  The PE-throttled bottleneck of v1 (one one-hot matmul per 128 messages)
  is gone: messages are gathered DIRECTLY INTO their destination lane via
  a host-computed permutation, so the segment sum is a plain DVE add of
  [128 lanes, tiles, 128 feat] blocks.  Per layer the PE only does the
  dense GEMM (49 transposes + 49 matmuls).

  - Nodes are permuted host-side: sorted by degree into 49 "bands"; band
    t supplies tile t of every core (128 lanes/core).  Degree-banding
    equalizes per-lane message counts, so the rectangular (lane x level)
    slot grid wastes little.
  - Each node is assigned a half (lane<64 -> table 0) by a greedy that
    balances, for every destination, its in-edges across halves (keeps
    per-(half,lane) slot counts near deg/2).  Table h is AllGathered from
    h_sent[lanes 64h:64h+64, :, :] — a rectangular partition-range DMA.
  - Slot stream per (half): for level k, for tiles t with K[h,t]>k, 128
    slots (lane-major).  Level-k slots for a run of tiles are contiguous,
    so one DVE tensor op accumulates a whole level: acc += mt_slice.
  - Empty slots gather a zeroed pad row of the table.
  - Self loops ride in the stream as ordinary slots (source = own row).
  - Bias enters as acc += crep (crep[d,t,f] = sqrt(deg) * b[f], host
    const), then the ACT epilogue computes relu(dinv^2 * acc) =
    a*relu(a*agg + b) = next layer's pre-scaled x.  Final layer:
    out = dinv * acc (ACT Copy).
"""

from contextlib import ExitStack

import numpy as np
import ml_dtypes

import concourse.bacc as bacc
import concourse.bass as bass
import concourse.mybir as mybir
from concourse.tile import TileContext
from concourse import library_config

BF16 = mybir.dt.bfloat16
F32 = mybir.dt.float32
I16 = mybir.dt.int16
P = 128
HL = 64          # lanes per half
TG = 4           # tiles per PSUM bank group (GEMM)


def preprocess(edge_index, n_nodes, n_cores=8, call_size=4096, seed=0):
    """Host-side graph preprocessing.

    Returns (sched, per_core_data, perm_info).
    perm_info: node -> (core, l) permutation plus inverse for unsharding.
    """
    src = np.asarray(edge_index[0], dtype=np.int64)
    dst = np.asarray(edge_index[1], dtype=np.int64)
    deg = (np.bincount(dst, minlength=n_nodes) + 1).astype(np.float32)
    dinv = (1.0 / np.sqrt(deg)).astype(np.float32)
    sqd = np.sqrt(deg).astype(np.float32)

    NT = (n_nodes + n_cores * P - 1) // (n_cores * P)
    S = NT * P                      # padded nodes per core
    NP = S * n_cores                # padded total
    n_dummy = NP - n_nodes

    # --- node permutation: degree-sorted bands; band t -> tile t ---
    order = np.argsort(-deg, kind="stable")        # real nodes, deg desc
    # node id -> (band, core, slot-in-(core,band))
    core_of = np.empty(NP, dtype=np.int64)
    band_of = np.empty(NP, dtype=np.int64)
    ranked = np.concatenate([order, np.arange(n_nodes, NP)])  # dummies last
    r = np.arange(NP)
    band_of[ranked] = r // (n_cores * P)
    core_of[ranked] = (r % (n_cores * P)) % n_cores

    # --- half balancing (greedy): exactly HL of each (core, band)'s 128
    # nodes to half 0, minimizing per-dst |c0-c1| ---
    # CSR by source over real edges
    e_order = np.argsort(src, kind="stable")
    s_sorted = src[e_order]
    d_sorted = dst[e_order]
    sptr = np.searchsorted(s_sorted, np.arange(n_nodes + 1))
    D = np.zeros(n_nodes, dtype=np.int32)          # c0 - c1 per dst
    quota = np.full((n_cores, NT, 2), HL, dtype=np.int32)
    half_of = np.empty(NP, dtype=np.int8)
    # process high out-degree first (they matter most)
    out_deg = sptr[1:] - sptr[:-1]
    proc = np.argsort(-out_deg, kind="stable")
    for u in proc:
        c, b = core_of[u], band_of[u]
        q0, q1 = quota[c, b, 0], quota[c, b, 1]
        if q0 == 0:
            h = 1
        elif q1 == 0:
            h = 0
        else:
            vs = d_sorted[sptr[u]:sptr[u + 1]]
            s = int(D[vs].sum()) if len(vs) else 0
            h = 1 if s > 0 else 0
        half_of[u] = h
        quota[c, b, h] -= 1
        if sptr[u + 1] > sptr[u]:
            vs = d_sorted[sptr[u]:sptr[u + 1]]
            np.add.at(D, vs, 1 - 2 * h)
    for u in range(n_nodes, NP):                   # dummies fill quotas
        c, b = core_of[u], band_of[u]
        h = 0 if quota[c, b, 0] > 0 else 1
        half_of[u] = h
        quota[c, b, h] -= 1
    assert (quota == 0).all()

    # --- quota-preserving swap refinement (minimize sum D^2 over dsts),
    # with incremental sD maintenance so decisions never go stale ---
    ed_order = np.argsort(d_sorted, kind="stable")   # edges sorted by dst
    in_src = s_sorted[ed_order]
    in_d = d_sorted[ed_order]
    in_ptr = np.searchsorted(in_d, np.arange(n_nodes + 1))
    from scipy.sparse import coo_matrix as _coo
    Adj = _coo((np.ones(len(src), np.int64), (src, dst)),
               shape=(n_nodes, n_nodes)).tocsr()
    sD = np.zeros(NP, np.int64)
    sD[:n_nodes] = Adj @ D.astype(np.int64)
    k_out = np.zeros(NP, np.int64)
    k_out[:n_nodes] = out_deg
    grp_id = core_of * NT + band_of                # node -> group
    grp_members = [np.nonzero(grp_id == g)[0] for g in range(n_cores * NT)]

    def _apply_flip(u, s):
        """Flip node u's half; D at its dsts changes by s (+-2)."""
        if u >= n_nodes:
            return
        vs = d_sorted[sptr[u]:sptr[u + 1]]
        D[vs] += s
        ins = np.concatenate([in_src[in_ptr[v]:in_ptr[v + 1]] for v in vs]) \
            if len(vs) else np.empty(0, np.int64)
        if len(ins):
            np.add.at(sD, ins, s)

    for _ in range(4):
        flipped = 0
        for g in range(n_cores * NT):
            members = grp_members[g]
            m0 = members[half_of[members] == 0]
            m1 = members[half_of[members] == 1]
            gain01 = sD[m0] - k_out[m0]            # flip 0->1 gain (x4)
            gain10 = -sD[m1] - k_out[m1]           # flip 1->0 gain
            c0 = m0[np.argsort(-gain01, kind="stable")]
            c1 = m1[np.argsort(-gain10, kind="stable")]
            g0 = np.sort(gain01)[::-1]
            g1 = np.sort(gain10)[::-1]
            for u0, u1, a, b in zip(c0, c1, g0, g1):
                if a + b <= 0:
                    break
                half_of[u0] = 1
                half_of[u1] = 0
                _apply_flip(u0, -2)
                _apply_flip(u1, 2)
                flipped += 2
        if flipped == 0:
            break

    # --- lane assignment: half0 -> lanes 0..63, half1 -> 64..127 ---
    lane_of = np.empty(NP, dtype=np.int64)
    nodes_by_cb = [[[] for _ in range(NT)] for _ in range(n_cores)]
    for u in ranked:                               # rank order within groups
        nodes_by_cb[core_of[u]][band_of[u]].append(u)
    for c in range(n_cores):
        for b in range(NT):
            grp = nodes_by_cb[c][b]
            assert len(grp) == P
            i0 = i1 = 0
            for u in grp:
                if half_of[u] == 0:
                    lane_of[u] = i0
                    i0 += 1
                else:
                    lane_of[u] = HL + i1
                    i1 += 1
            assert i0 == HL and i1 == HL

    l_of = band_of * P + lane_of                   # local row index
    # inverse permutation for output unshard: out row of node u
    perm_rows = core_of * S + l_of                 # node -> global padded row

    # --- table row index of each node (as source) ---
    RS = NT * HL + HL                              # shard rows per half (+pad)
    ZROW = NT * HL                                 # zero rows at shard tail
    tbl_row = core_of * RS + band_of * HL + (lane_of % HL)

    # --- per-destination source lists (table rows), split by src half ---
    # (self loops are added on-device as a direct DVE add of h_sent)
    msrc = src
    mdst = dst
    mh = half_of[msrc].astype(np.int64)
    mrow = tbl_row[msrc]
    # group by (dst, half): sort by (dst, half, mrow)
    key = (mdst * 2 + mh) * (RS * n_cores + 1) + mrow
    g_order = np.argsort(key, kind="stable")
    gd = mdst[g_order]
    gh = mh[g_order]
    grow = mrow[g_order]
    # counts per (dst, half)
    cnt = np.zeros((NP, 2), dtype=np.int64)
    np.add.at(cnt, (gd, gh), 1)
    # start offset of each (dst, half) run in the sorted stream
    run_len = cnt.reshape(-1)
    run_start = np.concatenate([[0], np.cumsum(run_len)[:-1]])

    # --- K levels per (half, tile): max over cores & lanes, monotone env ---
    K = np.zeros((2, NT), dtype=np.int64)
    cview = cnt.reshape(NP, 2)
    for h in range(2):
        per_node = cview[:, h]
        # per (band) max over all nodes in that band (all cores)
        Kt = np.zeros(NT, dtype=np.int64)
        np.maximum.at(Kt, band_of, per_node)
        K[h] = Kt
    for h in range(2):                             # monotone non-increasing
        for t in range(NT - 2, -1, -1):
            K[h, t] = max(K[h, t], K[h, t + 1])

    # level prefix sizes T[h][k] = #tiles with K[h,t] > k
    T_lvl = [[int((K[h] > k).sum()) for k in range(int(K[h].max()))]
             for h in range(2)]

    # --- build per-core idx streams (k-level-major) ---
    ZIDX = ZROW                                    # zero row of core 0
    node_at = np.empty(NP, dtype=np.int64)         # (c*S + l) -> node id
    node_at[perm_rows] = np.arange(NP)
    L0 = sum(T_lvl[0]) * P
    L1 = sum(T_lvl[1]) * P
    LT = L0 + L1
    per_core = []
    for c in range(n_cores):
        idx_stream = np.full(LT, ZIDX, dtype=np.int64)
        pos = 0
        for h in range(2):
            for k, Tk in enumerate(T_lvl[h]):
                nodes = node_at[c * S + np.arange(Tk * P)]   # (t,d) row-major
                starts = run_start[nodes * 2 + h]
                lens = run_len[nodes * 2 + h]
                sel = lens > k
                idx_stream[pos + np.nonzero(sel)[0]] = grow[starts[sel] + k]
                pos += Tk * P
        assert pos == LT
        idxw = idx_stream.astype(np.int16).reshape(LT // 16, 16).T
        idxw = np.tile(idxw, (8, 1))
        per_core.append(dict(idxw=np.ascontiguousarray(idxw)))

    # --- per-core constants (lane-major) ---
    ids = np.arange(S)
    for c in range(n_cores):
        nods = node_at[c * S + ids]
        dvc = np.zeros((P, NT), dtype=np.float32)
        dv2 = np.zeros((P, NT), dtype=np.float32)
        sq = np.zeros((P, NT), dtype=np.float32)
        real = nods < n_nodes
        dvc[ids[real] % P, ids[real] // P] = dinv[nods[real]]
        dv2[ids[real] % P, ids[real] // P] = dinv[nods[real]] ** 2
        sq[ids[real] % P, ids[real] // P] = sqd[nods[real]]
        per_core[c].update(dinvc=dvc, dinv2c=dv2, sqdc=sq)

    # --- gather calls ---
    calls = []   # (half, start, n)
    for h, (lo, ln) in enumerate(((0, L0), (L0, L1))):
        off = 0
        while off < ln:
            n = min(call_size, ln - off)
            calls.append((h, lo + off, n))
            off += n

    # --- add segments: (half, level, tile0, ntiles, start_slot) ---
    segs = []
    pos = 0
    for h in range(2):
        half_lo = 0 if h == 0 else L0
        for k, Tk in enumerate(T_lvl[h]):
            # split [0, Tk) tile range at call boundaries (grid per half)
            t0 = 0
            while t0 < Tk:
                rel = pos + t0 * P - half_lo
                room = (rel // call_size + 1) * call_size - rel
                t1 = min(Tk, t0 + room // P)
                segs.append((h, k, t0, t1 - t0, pos + t0 * P))
                t0 = t1
            pos += Tk * P
    assert pos == LT
    assert T_lvl[0][0] == NT and T_lvl[1][0] == NT, \
        "level 0 of each half must cover all tiles"

    sched = dict(n_nodes=n_nodes, n_cores=n_cores, S=S, NT=NT, RS=RS,
                 K=K, T_lvl=T_lvl, L0=L0, L1=L1, LT=LT, calls=calls,
                 segs=segs, call_size=call_size)
    perm_info = dict(perm_rows=perm_rows, node_at=node_at, dinv=dinv,
                     sqd=sqd, n_dummy=n_dummy)
    return sched, per_core, perm_info


def build_nc(sched):
    """Build the SPMD Bass graph (identical for all 8 cores).

    v3: table lives in SBUF (SRAM random access — no HBM row-miss cost on
    the gather); gather runs in transpose mode so messages, accumulator
    and x are all FEATURE-major.  Self-loop = W^T @ xT matmul (also
    initializes acc).  No PE transposes except the final output.
    """
    S, NT, RS = sched["S"], sched["NT"], sched["RS"]
    calls, segs = sched["calls"], sched["segs"]
    n_cores = sched["n_cores"]
    call_size = sched["call_size"]
    LT, L0 = sched["LT"], sched["L0"]
    TBL = RS * n_cores
    TTI = TBL // P                   # table tiles in SBUF
    core_ids = list(range(n_cores))

    nc = bacc.Bacc("TRN2", target_bir_lowering=False, num_devices=n_cores,
                   num_swdge_queues=4)

    x_in = nc.dram_tensor("x", [P, S], BF16, kind="ExternalInput")  # xT
    w_in = [nc.dram_tensor(f"w{i+1}", [P, P], BF16, kind="ExternalInput")
            for i in range(3)]
    crep_in = nc.dram_tensor("crep", [P, 3, S], BF16, kind="ExternalInput")
    a2t_in = nc.dram_tensor("a2t", [P, S], BF16, kind="ExternalInput")
    at_in = nc.dram_tensor("at", [P, S], BF16, kind="ExternalInput")
    ident_in = nc.dram_tensor("identb", [P, P], BF16, kind="ExternalInput")
    idxw_in = nc.dram_tensor("idxw", [P, LT // 16], I16, kind="ExternalInput")
    out_ext = nc.dram_tensor("out", [S, 64], F32, kind="ExternalOutput")

    with TileContext(nc) as tc, ExitStack() as ex:
        const = ex.enter_context(tc.tile_pool(name="const", bufs=1))
        dram = ex.enter_context(tc.tile_pool(name="dram", bufs=1, space="DRAM"))
        sb = ex.enter_context(tc.tile_pool(name="sb", bufs=2))
        xnp = ex.enter_context(tc.tile_pool(name="xnp", bufs=1))
        tblp = ex.enter_context(tc.tile_pool(name="tblp", bufs=1))
        msgp = ex.enter_context(tc.tile_pool(name="msgp", bufs=6))
        crepp = ex.enter_context(tc.tile_pool(name="crepp", bufs=2))
        accp = ex.enter_context(tc.tile_pool(name="accp", bufs=1))
        outp = ex.enter_context(tc.tile_pool(name="outp", bufs=2))
        ps_gemm = ex.enter_context(tc.tile_pool(name="ps_gemm", bufs=2, space="PSUM"))
        ps_self = ex.enter_context(tc.tile_pool(name="ps_self", bufs=2, space="PSUM"))
        ps_tr = ex.enter_context(tc.tile_pool(name="ps_tr", bufs=2, space="PSUM"))

        nc.gpsimd.load_library(library_config.mlp)

        def load_const(name, src_ap, shape, dtype):
            t = const.tile(shape, dtype, name=name)
            nc.sync.dma_start(t[:], src_ap)
            return t

        w_sb = [load_const(f"w{i}", w_in[i][:], [P, P], BF16) for i in range(3)]
        identb = load_const("identb", ident_in[:], [P, P], BF16)
        idxw = load_const("idxw", idxw_in[:], [P, LT // 16], I16)
        zeros64 = const.tile([HL, P], BF16, name="zeros64")
        nc.gpsimd.memset(zeros64[:], 0.0)
        sct = const.tile([P, S], BF16, name="sct")   # a^2 (layers 0-1), a (2)
        nc.sync.dma_start(sct[:], a2t_in[:])

        x_cur = xnp.tile([P, S], BF16, name="x_next")  # xT, feature-major
        nc.sync.dma_start(x_cur[:], x_in[:])
        g_gather = 0   # keeps queue_num aligned with Tile's DMASW lanes

        for layer in range(3):

            # ---- GEMM h_sent[node,f] = xT^T @ W ; self/init accT = W^T@xT
            h_sent = sb.tile([P, NT, P], BF16, name="h_sent")
            acc = accp.tile([P, S], F32, name="acc")    # DVE accumulator
            acc2 = accp.tile([P, S], F32, name="acc2")  # GPSIMD accumulator
            for g in range(0, NT, TG):
                gsz = min(TG, NT - g)
                g_ps = ps_gemm.tile([P, TG, P], F32, space="PSUM", name="g_ps")
                s_ps = ps_self.tile([P, TG, P], F32, space="PSUM", name="s_ps")
                for j in range(gsz):
                    t = g + j
                    xt_t = x_cur[:, t * P:(t + 1) * P]
                    nc.tensor.matmul(out=g_ps[:, j, :], lhsT=xt_t,
                                     rhs=w_sb[layer][:], start=True, stop=True)
                    nc.tensor.matmul(out=s_ps[:, j, :], lhsT=w_sb[layer][:],
                                     rhs=xt_t, start=True, stop=True)
                nc.vector.tensor_copy(h_sent[:, g:g + gsz, :], g_ps[:, :gsz, :])
                # acc starts as self-contribution + bias row
                crep = crepp.tile([P, TG * P], BF16, name="crep")
                nc.sync.dma_start(crep[:, :gsz * P],
                                  crep_in[:, layer, g * P:(g + gsz) * P])
                nc.vector.tensor_tensor(
                    out=acc[:, g * P:(g + gsz) * P],
                    in0=s_ps[:, :gsz, :].rearrange("p t f -> p (t f)"),
                    in1=crep[:, :gsz * P],
                    op=mybir.AluOpType.add)

            # ---- bounce (lane-halves) -> DRAM -> 2 AllGathers ----
            bounce = dram.tile([2 * RS, P], BF16, name="bounce")
            tbls = []
            for h in range(2):
                nc.sync.dma_start(
                    bounce[h * RS:h * RS + NT * HL, :].rearrange(
                        "(t p) f -> p t f", p=HL),
                    h_sent[h * HL:(h + 1) * HL, :, :])
                nc.sync.dma_start(
                    bounce[h * RS + NT * HL:(h + 1) * RS, :].rearrange(
                        "(t p) f -> p t f", t=1),
                    zeros64[:].rearrange("p (t f) -> p t f", t=1))
                tbl = dram.tile([TBL, P], BF16, addr_space="Shared",
                                name=f"tbl{h}")
                nc.gpsimd.collective_compute(
                    "AllGather", mybir.AluOpType.bypass,
                    replica_groups=[core_ids],
                    ins=[bounce[h * RS:(h + 1) * RS, :]],
                    outs=[tbl[:]])
                tbls.append(tbl)

            # ---- per half: table -> SBUF, SBUF-source transpose gathers.
            # Level adds: half 0 -> DVE into acc; half 1 -> GPSIMD into
            # acc2 (parallel engines, independent accumulator chains).
            msg_tiles = {}
            seg_i = 0
            for h in range(2):
                stbl = tblp.tile([P, TTI, P], BF16, name="stbl")
                nc.sync.dma_start(
                    stbl[:], tbls[h][:].rearrange("(t p) f -> p t f", p=P))
                for (hh, start, n) in calls:
                    if hh != h:
                        continue
                    mt = msgp.tile([P, 1, call_size], BF16, name="mt")
                    nc.gpsimd.dma_gather(
                        mt[:, :, 0:n], stbl[:].rearrange("p t f -> p (t f)"),
                        idxw[:, start // 16:(start + n) // 16],
                        n, n, P, transpose=True,
                        sbuf_tokens_per_rank=P,
                        sbuf_free_dim_per_rank=P * 2,
                        queue_num=g_gather % 4)
                    g_gather += 1
                    msg_tiles[start] = (start, n, mt)
                # level adds for this half (segs are in slot order)
                while seg_i < len(segs):
                    (hh, k, t0, ntl, slot) = segs[seg_i]
                    if hh != h:
                        break
                    cs = None
                    for s0, (cs0, cn0, mt0) in msg_tiles.items():
                        if cs0 <= slot < cs0 + cn0:
                            cs, mt = cs0, mt0
                    ms = slot - cs
                    src = mt[:, 0, ms:ms + ntl * P]
                    dst_lo, dst_hi = t0 * P, (t0 + ntl) * P
                    if h == 0:
                        nc.vector.tensor_tensor(
                            out=acc[:, dst_lo:dst_hi],
                            in0=acc[:, dst_lo:dst_hi], in1=src,
                            op=mybir.AluOpType.add)
                    elif k == 0:
                        nc.gpsimd.tensor_copy(acc2[:, dst_lo:dst_hi], src)
                    else:
                        nc.gpsimd.tensor_tensor(
                            out=acc2[:, dst_lo:dst_hi],
                            in0=acc2[:, dst_lo:dst_hi], in1=src,
                            op=mybir.AluOpType.add)
                    seg_i += 1
            assert seg_i == len(segs)
            # merge the two accumulators
            nc.vector.tensor_tensor(out=acc[:], in0=acc[:], in1=acc2[:],
                                    op=mybir.AluOpType.add)

            # ---- epilogue (feature-major) ----
            if layer == 2:
                nc.sync.dma_start(sct[:], at_in[:])
            nc.vector.tensor_tensor(
                out=acc[:], in0=acc[:], in1=sct[:], op=mybir.AluOpType.mult)
            if layer < 2:
                xn = sb.tile([P, S], BF16, name="x_next")
                for g in range(0, NT, TG):
                    gsz = min(TG, NT - g)
                    nc.scalar.activation(
                        out=xn[:, g * P:(g + gsz) * P],
                        in_=acc[:, g * P:(g + gsz) * P],
                        func=mybir.ActivationFunctionType.Relu,
                        scale=1.0)
                x_cur = xn
            else:
                for g in range(0, NT, TG):
                    gsz = min(TG, NT - g)
                    og = outp.tile([P, TG, 64], F32, name="og")
                    for j in range(gsz):
                        t = g + j
                        accb = outp.tile([P, P], BF16, name="accb")
                        nc.vector.tensor_copy(accb[:],
                                              acc[:, t * P:(t + 1) * P])
                        tr_ps = ps_tr.tile([P, P], BF16, space="PSUM",
                                           name="tr_ps")
                        nc.tensor.transpose(out=tr_ps[:], in_=accb[:],
                                            identity=identb[:])
                        nc.vector.tensor_copy(og[:, j, :], tr_ps[:, :64])
                    nc.sync.dma_start(
                        out_ext[g * P:(g + gsz) * P, :].rearrange(
                            "(t p) f -> p t f", p=P),
                        og[:, :gsz, :])

    nc.compile()
    return nc


def make_in_maps(x, W1, b1, W2, b2, W3, b3, sched, per_core, perm_info):
    """Build per-core input dicts (x permuted, pre-scaled, TRANSPOSED)."""
    S, NT = sched["S"], sched["NT"]
    n_cores = sched["n_cores"]
    n_nodes = sched["n_nodes"]
    bf = ml_dtypes.bfloat16
    w1 = np.asarray(W1, np.float32).astype(bf)
    w2 = np.asarray(W2, np.float32).astype(bf)
    w3 = np.zeros((P, P), np.float32)
    w3[:, :64] = np.asarray(W3, np.float32)
    w3 = w3.astype(bf)
    identb = np.eye(P, dtype=np.float32).astype(bf)
    dinv = perm_info["dinv"]
    node_at = perm_info["node_at"]
    xs = np.asarray(x, np.float32) * dinv[:, None]
    perm_rows = perm_info["perm_rows"]
    xp_all = np.zeros((n_cores * S, P), np.float32)
    xp_all[perm_rows[:n_nodes]] = xs

    bs = [np.asarray(b1, np.float32),
          np.asarray(b2, np.float32),
          np.zeros(P, np.float32)]
    bs[2][:64] = np.asarray(b3, np.float32)

    dinv_l = np.zeros(n_cores * S, np.float32)   # per padded row
    sqd_l = np.zeros(n_cores * S, np.float32)
    real = node_at < n_nodes
    dinv_l[real] = dinv[node_at[real]]
    sqd_l[real] = perm_info["sqd"][node_at[real]]

    in_maps = []
    for c in range(n_cores):
        d = per_core[c]
        sq = sqd_l[c * S:(c + 1) * S]            # [S]
        dv = dinv_l[c * S:(c + 1) * S]
        crep = np.empty((P, 3, S), np.float32)
        for li in range(3):
            crep[:, li, :] = bs[li][:, None] * sq[None, :]
        in_maps.append({
            "x": np.ascontiguousarray(xp_all[c * S:(c + 1) * S].T).astype(bf),
            "w1": w1, "w2": w2, "w3": w3,
            "crep": crep.astype(bf),
            "a2t": np.broadcast_to(dv * dv, (P, S)).astype(bf),
            "at": np.broadcast_to(dv, (P, S)).astype(bf),
            "identb": identb,
            "idxw": np.ascontiguousarray(d["idxw"]),
        })
    return in_maps


def unshard_output(res_outs, sched, perm_info):
    """Concatenate per-core outputs and un-permute to node order."""
    n_cores = sched["n_cores"]
    n_nodes = sched["n_nodes"]
    full = np.concatenate([np.asarray(res_outs[c]) for c in range(n_cores)],
                          axis=0)
    return full[perm_info["perm_rows"][:n_nodes]]


# ---------------------------------------------------------------------------
N_NODES = 50000
N_CORES = 8
CALL_SIZE = 512


def _run(inputs, trace=False):
    from concourse.bass_utils import run_bass_kernel_spmd

    x = np.asarray(inputs["x"], np.float32)
    edge_index = np.asarray(inputs["edge_index"])
    sched, per_core, perm_info = preprocess(edge_index, N_NODES, N_CORES,
                                            CALL_SIZE)
    nc = build_nc(sched)
    in_maps = make_in_maps(x, inputs["W1"], inputs["b1"], inputs["W2"],
                           inputs["b2"], inputs["W3"], inputs["b3"],
                           sched, per_core, perm_info)
    res = run_bass_kernel_spmd(nc, in_maps, list(range(N_CORES)), trace=trace)
    out = unshard_output([res.results[c]["out"] for c in range(N_CORES)],
                         sched, perm_info)
    return out.astype(np.float32), res


def kernel(x, edge_index, W1, b1, W2, b2, W3, b3):
    out, _ = _run(dict(x=x, edge_index=edge_index, W1=W1, b1=b1,
                       b2=b2, W2=W2, W3=W3, b3=b3), trace=False)
    return out


# revision 47
# speedup vs baseline: 4.1358x; 4.1358x over previous
"""GCN (3-layer) Bass kernel for 8 TRN2 NeuronCores, SPMD.

Math: out = A_hat @ relu(A_hat @ relu(A_hat @ X W1 + b1) W2 + b2) W3 + b3
where A_hat = D^-1/2 (A + I) D^-1/2 (in-degree over col of edge_index + self loop).

Factorization (per-node scaling absorbs the symmetric norm, a = dinv > 0):
  Xs = a * X                      (host-side for layer 1)
  H_sent = Xs @ W = a * (X @ W)   (GEMM epilogue is a pure f32->bf16 cast)
  Agg[c] = sum_{e: dst=c, src!=c} H_sent[src] + H_sent[c]    (self loop = PE
           identity matmul, cross edges = dma_gather + one-hot S matmuls)
  Xs_next = a * relu(a * Agg + b) = relu(a^2 * Agg + a*b)    (one ACT op,
           bias enters via a per-tile "fake block": lhsT row0 = sqrt(deg),
           rhs row0 = b, so the a^2 scale turns it into a*b)
  out (layer 3) = a * Agg + b     (ACT Copy, scale = a)

Layout per core (core owns S = N/8 contiguous dst nodes, NT = ceil(S/128)
tiles of 128):
  - GEMM on local shard -> bounce DRAM -> 2 AllGathers (top/bottom
    half-shards) -> two gather tables of TBL = N/2 rows each (so int16
    indices stay in range).
  - Cross-edge messages sorted by (src-half, dst_tile, src); each (half,tile)
    run padded to a multiple of 128 with dummy slots (idx=0, dstloc=-1).
    Block counts = max over the 8 cores (single SPMD instruction stream).
  - dma_gather (4 SWDGE queues round-robin -> ~2.5x descriptor-gen
    parallelism) pulls 256B bf16 rows message-major:
    msg[p, c, :] = row of message c*128+p.
  - S built 8 blocks per DVE op: S[p, k, j] = (dstloc[p, k] == j) via
    tensor_tensor is_equal against an iota row broadcast.
  - PE: psum[dst, feat] += S_k^T @ msg_k, one PSUM bank = 4 dst tiles.
"""

from contextlib import ExitStack

import numpy as np
import ml_dtypes

import concourse.bacc as bacc
import concourse.bass as bass
import concourse.mybir as mybir
from concourse.tile import TileContext
from concourse import library_config

BF16 = mybir.dt.bfloat16
F32 = mybir.dt.float32
I16 = mybir.dt.int16
P = 128
SBK = 16         # S-build batch (blocks per DVE op)
TG = 4           # dst tiles per PSUM bank group


def preprocess(edge_index, n_nodes, n_cores=8, call_size=1792):
    """Host-side index preprocessing. Returns (sched, per_core_data, dinv)."""
    src = np.asarray(edge_index[0], dtype=np.int64)
    dst = np.asarray(edge_index[1], dtype=np.int64)
    deg = (np.bincount(dst, minlength=n_nodes) + 1).astype(np.float32)
    dinv = 1.0 / np.sqrt(deg)
    sqd = np.sqrt(deg)

    S = n_nodes // n_cores
    assert S * n_cores == n_nodes and S % 2 == 0
    HS = S // 2
    TBL = HS * n_cores
    NT = (S + P - 1) // P

    core_of = dst // S
    shard = src // S
    sloc = src % S
    half = (sloc >= HS).astype(np.int64)
    trow = shard * HS + (sloc - half * HS)
    dloc = dst % S
    dtile = dloc // P
    dlane = dloc % P

    counts = np.zeros((n_cores, 2, NT), dtype=np.int64)
    for c in range(n_cores):
        m = core_of == c
        np.add.at(counts[c], (half[m], dtile[m]), 1)
    B = np.ceil(counts / P).max(axis=0).astype(np.int64)  # [2, NT]

    per_core = []
    for c in range(n_cores):
        m = core_of == c
        h_c, t_c, r_c, l_c = half[m], dtile[m], trow[m], dlane[m]
        idx_stream = []
        dl_stream = []
        for hh in range(2):
            for tt in range(NT):
                sel = (h_c == hh) & (t_c == tt)
                rows = r_c[sel]
                lanes = l_c[sel]
                order = np.argsort(rows, kind="stable")  # DMA locality
                rows, lanes = rows[order], lanes[order]
                pad = B[hh, tt] * P - len(rows)
                assert pad >= 0
                idx_stream.append(rows)
                idx_stream.append(np.zeros(pad, dtype=np.int64))
                dl_stream.append(lanes)
                dl_stream.append(np.full(pad, -1, dtype=np.int64))
        idx_stream = np.concatenate(idx_stream)
        dl_stream = np.concatenate(dl_stream)
        LT = len(idx_stream)
        assert LT == B.sum() * P
        idxw = idx_stream.astype(np.int16).reshape(LT // 16, 16).T
        idxw = np.tile(idxw, (8, 1))
        dl = dl_stream.reshape(LT // P, P).T.astype(np.float32)
        dvc = np.zeros((P, NT), dtype=np.float32)
        dv2 = np.zeros((P, NT), dtype=np.float32)
        sfk = np.zeros((P, NT * P), dtype=np.float32)
        ids = np.arange(S)
        dvc[ids % P, ids // P] = dinv[c * S + ids]
        dv2[ids % P, ids // P] = dinv[c * S + ids] ** 2
        sfk[0, ids] = sqd[c * S + ids]
        per_core.append(dict(idxw=idxw, dstloc=dl, dinvc=dvc, dinv2c=dv2,
                             sfake=sfk.astype(ml_dtypes.bfloat16)))

    L0 = int(B[0].sum()) * P
    L1 = int(B[1].sum()) * P
    calls = []  # (phase, start_msg, n_msgs)
    for hh, (lo, ln) in enumerate(((0, L0), (L0, L1))):
        off = 0
        while off < ln:
            n = min(call_size, ln - off)
            calls.append((hh, lo + off, n))
            off += n

    sched = dict(n_nodes=n_nodes, n_cores=n_cores, S=S, HS=HS, TBL=TBL, NT=NT,
                 B=B, L0=L0, L1=L1, calls=calls, call_size=call_size)
    return sched, per_core, dinv


def build_nc(sched):
    """Build the SPMD Bass graph (identical for all 8 cores)."""
    S, HS, TBL, NT = sched["S"], sched["HS"], sched["TBL"], sched["NT"]
    B, calls = sched["B"], sched["calls"]
    n_cores = sched["n_cores"]
    call_size = sched["call_size"]
    NB = int(B.sum())
    LT = NB * P
    core_ids = list(range(n_cores))
    n_full = S // P
    rem = S - n_full * P

    nc = bacc.Bacc("TRN2", target_bir_lowering=False, num_devices=n_cores,
                   num_swdge_queues=4)

    x_in = nc.dram_tensor("x", [NT * P, P], F32, kind="ExternalInput")
    w_in = [nc.dram_tensor(f"w{i+1}", [P, P], BF16, kind="ExternalInput")
            for i in range(3)]
    bfake_in = nc.dram_tensor("bfake", [P, 3, P], BF16, kind="ExternalInput")
    sfake_in = nc.dram_tensor("sfake", [P, NT * P], BF16, kind="ExternalInput")
    dinv_in = nc.dram_tensor("dinvc", [P, NT], F32, kind="ExternalInput")
    dinv2_in = nc.dram_tensor("dinv2c", [P, NT], F32, kind="ExternalInput")
    ident_in = nc.dram_tensor("identb", [P, P], BF16, kind="ExternalInput")
    iota_in = nc.dram_tensor("iotab", [P, SBK * P], BF16, kind="ExternalInput")
    idxw_in = nc.dram_tensor("idxw", [P, LT // 16], I16, kind="ExternalInput")
    dstloc_in = nc.dram_tensor("dstloc", [P, NB], F32, kind="ExternalInput")
    out_ext = nc.dram_tensor("out", [S, 64], F32, kind="ExternalOutput")

    with TileContext(nc) as tc, ExitStack() as ex:
        const = ex.enter_context(tc.tile_pool(name="const", bufs=1))
        dram = ex.enter_context(tc.tile_pool(name="dram", bufs=1, space="DRAM"))
        sb = ex.enter_context(tc.tile_pool(name="sb", bufs=2))
        msgp = ex.enter_context(tc.tile_pool(name="msgp", bufs=10))
        spool = ex.enter_context(tc.tile_pool(name="spool", bufs=4))
        xtp = ex.enter_context(tc.tile_pool(name="xtp", bufs=2))
        accp = ex.enter_context(tc.tile_pool(name="accp", bufs=1))
        ps_agg = ex.enter_context(tc.tile_pool(name="ps_agg", bufs=3, space="PSUM"))
        ps_gemm = ex.enter_context(tc.tile_pool(name="ps_gemm", bufs=2, space="PSUM"))
        ps_tr = ex.enter_context(tc.tile_pool(name="ps_tr", bufs=2, space="PSUM"))

        nc.gpsimd.load_library(library_config.mlp)


        def load_const(name, src_ap, shape, dtype):
            t = const.tile(shape, dtype, name=name)
            nc.sync.dma_start(t[:], src_ap)
            return t

        w_sb = [load_const(f"w{i}", w_in[i][:], [P, P], BF16) for i in range(3)]
        bfake = load_const("bfake", bfake_in[:], [P, 3, P], BF16)
        sfake = load_const("sfake", sfake_in[:], [P, NT * P], BF16)
        dinvc = load_const("dinvc", dinv_in[:], [P, NT], F32)
        dinv2c = load_const("dinv2c", dinv2_in[:], [P, NT], F32)
        identb = load_const("identb", ident_in[:], [P, P], BF16)
        iotab = load_const("iotab", iota_in[:], [P, SBK * P], BF16)
        idxw = load_const("idxw", idxw_in[:], [P, LT // 16], I16)
        dstloc = load_const("dstloc", dstloc_in[:], [P, NB], F32)

        x_prev = None  # SBUF [P, NT, P] bf16 = a*X for layers 2,3

        for layer in range(3):
            # ---- GEMM: h_sent = (a*X) @ W, pure-cast epilogue, 4-tile groups
            h_sent = sb.tile([P, NT, P], BF16, name="h_sent")
            for g in range(0, NT, TG):
                gsz = min(TG, NT - g)
                if layer == 0:
                    xf = sb.tile([P, TG, P], F32, name="xf")
                    nc.sync.dma_start(
                        xf[:, :gsz, :],
                        x_in[g * P:(g + gsz) * P, :].rearrange(
                            "(t p) f -> p t f", p=P))
                    xb = sb.tile([P, TG, P], BF16, name="xb")
                    nc.vector.tensor_copy(xb[:, :gsz, :], xf[:, :gsz, :])
                g_ps = ps_gemm.tile([P, TG, P], F32, space="PSUM", name="g_ps")
                for j in range(gsz):
                    t = g + j
                    xbj = xb[:, j, :] if layer == 0 else x_prev[:, t, :]
                    tr_ps = ps_tr.tile([P, P], BF16, space="PSUM", name="tr_ps")
                    nc.tensor.transpose(out=tr_ps[:], in_=xbj, identity=identb[:])
                    xt = xtp.tile([P, P], BF16, name="xt")
                    nc.vector.tensor_copy(xt[:], tr_ps[:])
                    nc.tensor.matmul(out=g_ps[:, j, :], lhsT=xt[:],
                                     rhs=w_sb[layer][:], start=True, stop=True)
                nc.vector.tensor_copy(h_sent[:, g:g + gsz, :], g_ps[:, :gsz, :])

            # ---- h_sent -> bounce DRAM -> two AllGathers ----
            bounce = dram.tile([S, P], BF16, name="bounce")

            def dma_rows(r0, r1):
                """DMA h_sent node rows [r0, r1) into bounce (tile-aligned
                middle as one big DMA, ragged edges separately)."""
                while r0 < r1:
                    t0, l0 = divmod(r0, P)
                    if l0 == 0 and r1 - r0 >= P:
                        tn = (r1 - r0) // P
                        nc.sync.dma_start(
                            bounce[r0:r0 + tn * P, :].rearrange(
                                "(t p) f -> p t f", p=P),
                            h_sent[:, t0:t0 + tn, :])
                        r0 += tn * P
                    else:
                        l1 = min(P, l0 + (r1 - r0))
                        nc.sync.dma_start(
                            bounce[r0:r0 + (l1 - l0), :].rearrange(
                                "(t p) f -> p t f", t=1),
                            h_sent[l0:l1, t0:t0 + 1, :])
                        r0 += l1 - l0

            dma_rows(0, HS)
            dma_rows(HS, S)
            tables = []
            for hh in range(2):
                tbl = dram.tile([TBL, P], BF16, addr_space="Shared",
                                name=f"tbl{hh}")
                nc.gpsimd.collective_compute(
                    "AllGather", mybir.AluOpType.bypass,
                    replica_groups=[core_ids],
                    ins=[bounce[hh * HS:(hh + 1) * HS, :]],
                    outs=[tbl[:]])
                tables.append(tbl)

            # ---- gather calls: prepare_only preps (desc-gen runs early,
            # hidden under the AllGather) + per-queue triggers that carry the
            # table dep.  Drains overlap across all 4 SWDGE queues.
            msg_tiles = []
            for ci, (hh, start, n) in enumerate(calls):
                mt = msgp.tile([P, call_size // P, P], BF16, name="mt")
                nc.gpsimd.dma_gather(
                    mt[:, 0:n // P, :], tables[hh][:],
                    idxw[:, start // 16:(start + n) // 16],
                    n, n, P, queue_num=ci % 4)
                msg_tiles.append((start, n, mt))

            def msg_ap(ms):
                for (cs, cn, mt) in msg_tiles:
                    if cs <= ms < cs + cn:
                        return mt[:, (ms - cs) // P, :]
                raise AssertionError

            # ---- batched S builds (SBK blocks per DVE op) ----
            s_tiles = []  # block index -> (tile, slot)
            for b0 in range(0, NB, SBK):
                k = min(SBK, NB - b0)
                st = spool.tile([P, SBK, P], BF16, name="st")
                nc.vector.tensor_tensor(
                    out=st[:, :k, :],
                    in0=iotab[:, 0:k * P].rearrange("p (k j) -> p k j", k=k),
                    in1=dstloc[:, b0:b0 + k].to_broadcast([P, k, P]),
                    op=mybir.AluOpType.is_equal)
                for j in range(k):
                    s_tiles.append((st, j))

            # ---- segment sum: per phase, 4-tile PSUM groups -> acc ----
            acc = accp.tile([P, NT, P], F32, name="acc")
            gb = 0
            boff = 0
            for hh in range(2):
                for g in range(0, NT, TG):
                    gsz = min(TG, NT - g)
                    a_ps = ps_agg.tile([P, TG, P], F32, space="PSUM", name="a_ps")
                    have = []  # slices written this phase
                    for j in range(gsz):
                        t = g + j
                        nb = int(B[hh, t])
                        if hh == 0:
                            nc.tensor.matmul(
                                out=a_ps[:, j, :],
                                lhsT=sfake[:, t * P:(t + 1) * P],
                                rhs=bfake[:, layer, :], start=True, stop=False)
                            nc.tensor.matmul(
                                out=a_ps[:, j, :], lhsT=identb[:],
                                rhs=h_sent[:, t, :], start=False, stop=(nb == 0))
                        elif nb == 0:
                            continue
                        have.append(j)
                        for i in range(nb):
                            st, slot = s_tiles[gb]
                            nc.tensor.matmul(
                                out=a_ps[:, j, :], lhsT=st[:, slot, :],
                                rhs=msg_ap(boff),
                                start=(hh == 1 and i == 0), stop=(i == nb - 1))
                            gb += 1
                            boff += P
                    if hh == 0:
                        nc.vector.tensor_copy(acc[:, g:g + gsz, :], a_ps[:, :gsz, :])
                    elif len(have) == gsz:
                        nc.vector.tensor_tensor(
                            out=acc[:, g:g + gsz, :], in0=acc[:, g:g + gsz, :],
                            in1=a_ps[:, :gsz, :], op=mybir.AluOpType.add)
                    else:
                        for j in have:
                            nc.vector.tensor_tensor(
                                out=acc[:, g + j, :], in0=acc[:, g + j, :],
                                in1=a_ps[:, j, :], op=mybir.AluOpType.add)

            # ---- epilogue ----
            if layer < 2:
                x_prev = sb.tile([P, NT, P], BF16, name="x_next")
                for t in range(NT):
                    nc.scalar.activation(
                        out=x_prev[:, t, :], in_=acc[:, t, :],
                        func=mybir.ActivationFunctionType.Relu,
                        scale=dinv2c[:, t:t + 1])
            else:
                out_sb = sb.tile([P, NT, 64], F32, name="out_sb")
                for t in range(NT):
                    nc.scalar.activation(
                        out=out_sb[:, t, :], in_=acc[:, t, :64],
                        func=mybir.ActivationFunctionType.Copy,
                        scale=dinvc[:, t:t + 1])
                if n_full:
                    nc.sync.dma_start(
                        out_ext[0:n_full * P, :].rearrange("(t p) f -> p t f", p=P),
                        out_sb[:, 0:n_full, :])
                if rem:
                    nc.sync.dma_start(
                        out_ext[n_full * P:S, :].rearrange("(t p) f -> p t f", t=1),
                        out_sb[0:rem, n_full:NT, :])

    nc.compile()
    return nc


def make_in_maps(x, W1, b1, W2, b2, W3, b3, sched, per_core, dinv):
    """Build per-core input dicts (x pre-scaled by dinv)."""
    S, NT = sched["S"], sched["NT"]
    n_cores = sched["n_cores"]
    bf = ml_dtypes.bfloat16
    w1 = np.asarray(W1, np.float32).astype(bf)
    w2 = np.asarray(W2, np.float32).astype(bf)
    w3 = np.zeros((P, P), np.float32)
    w3[:, :64] = np.asarray(W3, np.float32)
    w3 = w3.astype(bf)
    bfake = np.zeros((P, 3, P), np.float32)
    bfake[0, 0, :] = np.asarray(b1, np.float32)
    bfake[0, 1, :] = np.asarray(b2, np.float32)
    bfake[0, 2, :64] = np.asarray(b3, np.float32)
    bfake = bfake.astype(bf)
    identb = np.eye(P, dtype=np.float32).astype(bf)
    iotab = np.tile(np.arange(P, dtype=np.float32), (P, SBK)).astype(bf)
    xs = np.asarray(x, np.float32) * np.asarray(dinv)[:, None]

    in_maps = []
    for c in range(n_cores):
        d = per_core[c]
        xp = np.zeros((NT * P, P), np.float32)
        xp[:S] = xs[c * S:(c + 1) * S]
        in_maps.append({
            "x": xp,
            "w1": w1, "w2": w2, "w3": w3,
            "bfake": bfake,
            "sfake": np.ascontiguousarray(d["sfake"]),
            "dinvc": np.ascontiguousarray(d["dinvc"]),
            "dinv2c": np.ascontiguousarray(d["dinv2c"]),
            "identb": identb, "iotab": iotab,
            "idxw": np.ascontiguousarray(d["idxw"]),
            "dstloc": np.ascontiguousarray(d["dstloc"]),
        })
    return in_maps


# ---------------------------------------------------------------------------
# Entry point: full inputs in, full output out.  Hardcoded problem shapes.
# ---------------------------------------------------------------------------
N_NODES = 50000
N_CORES = 8
CALL_SIZE = 1024


def _run(inputs, trace=False):
    from concourse.bass_utils import run_bass_kernel_spmd

    x = np.asarray(inputs["x"], np.float32)
    edge_index = np.asarray(inputs["edge_index"])
    sched, per_core, dinv = preprocess(edge_index, N_NODES, N_CORES, CALL_SIZE)
    nc = build_nc(sched)
    in_maps = make_in_maps(x, inputs["W1"], inputs["b1"], inputs["W2"],
                           inputs["b2"], inputs["W3"], inputs["b3"],
                           sched, per_core, dinv)
    res = run_bass_kernel_spmd(nc, in_maps, list(range(N_CORES)), trace=trace)
    out = np.concatenate([np.asarray(res.results[c]["out"])
                          for c in range(N_CORES)], axis=0)
    return out.astype(np.float32), res


def kernel(x, edge_index, W1, b1, W2, b2, W3, b3):
    out, _ = _run(dict(x=x, edge_index=edge_index, W1=W1, b1=b1, W2=W2,
                       b2=b2, W3=W3, b3=b3), trace=False)
    return out



# revision 51
# speedup vs baseline: 4.1494x; 1.0033x over previous
"""GCN (3-layer) Bass kernel for 8 TRN2 NeuronCores, SPMD.

Math: out = A_hat @ relu(A_hat @ relu(A_hat @ X W1 + b1) W2 + b2) W3 + b3
where A_hat = D^-1/2 (A + I) D^-1/2 (in-degree over col of edge_index + self loop).

Factorization (per-node scaling absorbs the symmetric norm, a = dinv > 0):
  Xs = a * X                      (host-side for layer 1)
  H_sent = Xs @ W = a * (X @ W)   (GEMM epilogue is a pure f32->bf16 cast)
  Agg[c] = sum_{e: dst=c, src!=c} H_sent[src] + H_sent[c]    (self loop = PE
           identity matmul, cross edges = dma_gather + one-hot S matmuls)
  Xs_next = a * relu(a * Agg + b) = relu(a^2 * Agg + a*b)    (one ACT op,
           bias enters via a per-tile "fake block": lhsT row0 = sqrt(deg),
           rhs row0 = b, so the a^2 scale turns it into a*b)
  out (layer 3) = a * Agg + b     (ACT Copy, scale = a)

Layout per core (core owns S = N/8 contiguous dst nodes, NT = ceil(S/128)
tiles of 128):
  - GEMM on local shard -> bounce DRAM -> 2 AllGathers (top/bottom
    half-shards) -> two gather tables of TBL = N/2 rows each (so int16
    indices stay in range).
  - Cross-edge messages sorted by (src-half, dst_tile, src); each (half,tile)
    run padded to a multiple of 128 with dummy slots (idx=0, dstloc=-1).
    Block counts = max over the 8 cores (single SPMD instruction stream).
  - dma_gather (4 SWDGE queues round-robin -> ~2.5x descriptor-gen
    parallelism) pulls 256B bf16 rows message-major:
    msg[p, c, :] = row of message c*128+p.
  - S built 8 blocks per DVE op: S[p, k, j] = (dstloc[p, k] == j) via
    tensor_tensor is_equal against an iota row broadcast.
  - PE: psum[dst, feat] += S_k^T @ msg_k, one PSUM bank = 4 dst tiles.
"""

from contextlib import ExitStack

import numpy as np
import ml_dtypes

import concourse.bacc as bacc
import concourse.bass as bass
import concourse.mybir as mybir
from concourse.tile import TileContext
from concourse import library_config

BF16 = mybir.dt.bfloat16
F32 = mybir.dt.float32
I16 = mybir.dt.int16
P = 128
SBK = 16         # S-build batch (blocks per DVE op)
TG = 4           # dst tiles per PSUM bank group


def preprocess(edge_index, n_nodes, n_cores=8, call_size=1792):
    """Host-side index preprocessing. Returns (sched, per_core_data, dinv)."""
    src = np.asarray(edge_index[0], dtype=np.int64)
    dst = np.asarray(edge_index[1], dtype=np.int64)
    deg = (np.bincount(dst, minlength=n_nodes) + 1).astype(np.float32)
    dinv = 1.0 / np.sqrt(deg)
    sqd = np.sqrt(deg)

    S = n_nodes // n_cores
    assert S * n_cores == n_nodes and S % 2 == 0
    HS = S // 2
    TBL = HS * n_cores
    NT = (S + P - 1) // P

    core_of = dst // S
    shard = src // S
    sloc = src % S
    half = (sloc >= HS).astype(np.int64)
    trow = shard * HS + (sloc - half * HS)
    dloc = dst % S
    dtile = dloc // P
    dlane = dloc % P

    counts = np.zeros((n_cores, 2, NT), dtype=np.int64)
    for c in range(n_cores):
        m = core_of == c
        np.add.at(counts[c], (half[m], dtile[m]), 1)
    B = np.ceil(counts / P).max(axis=0).astype(np.int64)  # [2, NT]

    per_core = []
    for c in range(n_cores):
        m = core_of == c
        h_c, t_c, r_c, l_c = half[m], dtile[m], trow[m], dlane[m]
        idx_stream = []
        dl_stream = []
        for hh in range(2):
            for tt in range(NT):
                sel = (h_c == hh) & (t_c == tt)
                rows = r_c[sel]
                lanes = l_c[sel]
                order = np.argsort(rows, kind="stable")  # DMA locality
                rows, lanes = rows[order], lanes[order]
                pad = B[hh, tt] * P - len(rows)
                assert pad >= 0
                idx_stream.append(rows)
                idx_stream.append(np.zeros(pad, dtype=np.int64))
                dl_stream.append(lanes)
                dl_stream.append(np.full(pad, -1, dtype=np.int64))
        idx_stream = np.concatenate(idx_stream)
        dl_stream = np.concatenate(dl_stream)
        LT = len(idx_stream)
        assert LT == B.sum() * P
        idxw = idx_stream.astype(np.int16).reshape(LT // 16, 16).T
        idxw = np.tile(idxw, (8, 1))
        dl = dl_stream.reshape(LT // P, P).T.astype(np.float32)
        dvc = np.zeros((P, NT), dtype=np.float32)
        dv2 = np.zeros((P, NT), dtype=np.float32)
        sfk = np.zeros((P, NT * P), dtype=np.float32)
        ids = np.arange(S)
        dvc[ids % P, ids // P] = dinv[c * S + ids]
        dv2[ids % P, ids // P] = dinv[c * S + ids] ** 2
        sfk[0, ids] = sqd[c * S + ids]
        per_core.append(dict(idxw=idxw, dstloc=dl, dinvc=dvc, dinv2c=dv2,
                             sfake=sfk.astype(ml_dtypes.bfloat16)))

    L0 = int(B[0].sum()) * P
    L1 = int(B[1].sum()) * P
    calls = []  # (phase, start_msg, n_msgs)
    for hh, (lo, ln) in enumerate(((0, L0), (L0, L1))):
        off = 0
        while off < ln:
            n = min(call_size, ln - off)
            calls.append((hh, lo + off, n))
            off += n

    sched = dict(n_nodes=n_nodes, n_cores=n_cores, S=S, HS=HS, TBL=TBL, NT=NT,
                 B=B, L0=L0, L1=L1, calls=calls, call_size=call_size)
    return sched, per_core, dinv


def build_nc(sched):
    """Build the SPMD Bass graph (identical for all 8 cores)."""
    S, HS, TBL, NT = sched["S"], sched["HS"], sched["TBL"], sched["NT"]
    B, calls = sched["B"], sched["calls"]
    n_cores = sched["n_cores"]
    call_size = sched["call_size"]
    NB = int(B.sum())
    LT = NB * P
    core_ids = list(range(n_cores))
    n_full = S // P
    rem = S - n_full * P

    nc = bacc.Bacc("TRN2", target_bir_lowering=False, num_devices=n_cores,
                   num_swdge_queues=4)

    x_in = nc.dram_tensor("x", [NT * P, P], F32, kind="ExternalInput")
    w_in = [nc.dram_tensor(f"w{i+1}", [P, P], BF16, kind="ExternalInput")
            for i in range(3)]
    bfake_in = nc.dram_tensor("bfake", [P, 3, P], BF16, kind="ExternalInput")
    sfake_in = nc.dram_tensor("sfake", [P, NT * P], BF16, kind="ExternalInput")
    dinv_in = nc.dram_tensor("dinvc", [P, NT], F32, kind="ExternalInput")
    dinv2_in = nc.dram_tensor("dinv2c", [P, NT], F32, kind="ExternalInput")
    ident_in = nc.dram_tensor("identb", [P, P], BF16, kind="ExternalInput")
    iota_in = nc.dram_tensor("iotab", [P, SBK * P], BF16, kind="ExternalInput")
    idxw_in = nc.dram_tensor("idxw", [P, LT // 16], I16, kind="ExternalInput")
    dstloc_in = nc.dram_tensor("dstloc", [P, NB], F32, kind="ExternalInput")
    out_ext = nc.dram_tensor("out", [S, 64], F32, kind="ExternalOutput")

    with TileContext(nc) as tc, ExitStack() as ex:
        const = ex.enter_context(tc.tile_pool(name="const", bufs=1))
        dram = ex.enter_context(tc.tile_pool(name="dram", bufs=1, space="DRAM"))
        sb = ex.enter_context(tc.tile_pool(name="sb", bufs=2))
        msgp = ex.enter_context(tc.tile_pool(name="msgp", bufs=10))
        spool = ex.enter_context(tc.tile_pool(name="spool", bufs=4))
        xtp = ex.enter_context(tc.tile_pool(name="xtp", bufs=2))
        accp = ex.enter_context(tc.tile_pool(name="accp", bufs=1))
        ps_agg = ex.enter_context(tc.tile_pool(name="ps_agg", bufs=3, space="PSUM"))
        ps_gemm = ex.enter_context(tc.tile_pool(name="ps_gemm", bufs=2, space="PSUM"))
        ps_tr = ex.enter_context(tc.tile_pool(name="ps_tr", bufs=2, space="PSUM"))

        nc.gpsimd.load_library(library_config.mlp)


        def load_const(name, src_ap, shape, dtype):
            t = const.tile(shape, dtype, name=name)
            nc.sync.dma_start(t[:], src_ap)
            return t

        w_sb = [load_const(f"w{i}", w_in[i][:], [P, P], BF16) for i in range(3)]
        bfake = load_const("bfake", bfake_in[:], [P, 3, P], BF16)
        sfake = load_const("sfake", sfake_in[:], [P, NT * P], BF16)
        dinvc = load_const("dinvc", dinv_in[:], [P, NT], F32)
        dinv2c = load_const("dinv2c", dinv2_in[:], [P, NT], F32)
        identb = load_const("identb", ident_in[:], [P, P], BF16)
        iotab = load_const("iotab", iota_in[:], [P, SBK * P], BF16)
        idxw = load_const("idxw", idxw_in[:], [P, LT // 16], I16)
        dstloc = load_const("dstloc", dstloc_in[:], [P, NB], F32)

        def emit_gemm_group(layer, g, gsz, x_tiles, h_dst):
            """PE transpose + matmul for tiles [g, g+gsz) -> h_dst."""
            g_ps = ps_gemm.tile([P, TG, P], F32, space="PSUM", name="g_ps")
            for j in range(gsz):
                tr_ps = ps_tr.tile([P, P], BF16, space="PSUM", name="tr_ps")
                nc.tensor.transpose(out=tr_ps[:], in_=x_tiles(g + j),
                                    identity=identb[:])
                xt = xtp.tile([P, P], BF16, name="xt")
                nc.vector.tensor_copy(xt[:], tr_ps[:])
                nc.tensor.matmul(out=g_ps[:, j, :], lhsT=xt[:],
                                 rhs=w_sb[layer][:], start=True, stop=True)
            nc.vector.tensor_copy(h_dst[:, g:g + gsz, :], g_ps[:, :gsz, :])

        def dma_rows(bounce, h_src, r0, r1):
            """DMA h_src node rows [r0, r1) into bounce (tile-aligned
            middle as one big DMA, ragged edges separately)."""
            while r0 < r1:
                t0, l0 = divmod(r0, P)
                if l0 == 0 and r1 - r0 >= P:
                    tn = (r1 - r0) // P
                    nc.sync.dma_start(
                        bounce[r0:r0 + tn * P, :].rearrange(
                            "(t p) f -> p t f", p=P),
                        h_src[:, t0:t0 + tn, :])
                    r0 += tn * P
                else:
                    l1 = min(P, l0 + (r1 - r0))
                    nc.sync.dma_start(
                        bounce[r0:r0 + (l1 - l0), :].rearrange(
                            "(t p) f -> p t f", t=1),
                        h_src[l0:l1, t0:t0 + 1, :])
                    r0 += l1 - l0

        def emit_ag(bounce, hh, name):
            tbl = dram.tile([TBL, P], BF16, addr_space="Shared", name=name)
            nc.gpsimd.collective_compute(
                "AllGather", mybir.AluOpType.bypass,
                replica_groups=[core_ids],
                ins=[bounce[hh * HS:(hh + 1) * HS, :]],
                outs=[tbl[:]])
            return tbl

        # ---- layer-0 prologue: GEMM from x input, bounce, AllGathers ----
        h_sent = sb.tile([P, NT, P], BF16, name="h_sent")
        for g in range(0, NT, TG):
            gsz = min(TG, NT - g)
            xf = sb.tile([P, TG, P], F32, name="xf")
            nc.sync.dma_start(
                xf[:, :gsz, :],
                x_in[g * P:(g + gsz) * P, :].rearrange("(t p) f -> p t f", p=P))
            xb = sb.tile([P, TG, P], BF16, name="xb")
            nc.vector.tensor_copy(xb[:, :gsz, :], xf[:, :gsz, :])
            emit_gemm_group(0, g, gsz, lambda t, xb=xb, g=g: xb[:, t - g, :],
                            h_sent)
        bounce = dram.tile([S, P], BF16, name="bounce")
        dma_rows(bounce, h_sent, 0, S)
        tables = [emit_ag(bounce, 0, "tbl0"), emit_ag(bounce, 1, "tbl1")]
        g_gather = 0   # global count keeps queue_num aligned with Tile's
                       # DMASW lane cycling (mod 8 -> mod 4)

        for layer in range(3):

            # ---- gather calls: prepare_only preps (desc-gen runs early,
            # hidden under the AllGather) + per-queue triggers that carry the
            # table dep.  Drains overlap across all 4 SWDGE queues.
            msg_tiles = []
            for (hh, start, n) in calls:
                mt = msgp.tile([P, call_size // P, P], BF16, name="mt")
                nc.gpsimd.dma_gather(
                    mt[:, 0:n // P, :], tables[hh][:],
                    idxw[:, start // 16:(start + n) // 16],
                    n, n, P, queue_num=g_gather % 4)
                g_gather += 1
                msg_tiles.append((start, n, mt))

            def msg_ap(ms):
                for (cs, cn, mt) in msg_tiles:
                    if cs <= ms < cs + cn:
                        return mt[:, (ms - cs) // P, :]
                raise AssertionError

            # ---- batched S builds (SBK blocks per DVE op) ----
            s_tiles = []  # block index -> (tile, slot)
            for b0 in range(0, NB, SBK):
                k = min(SBK, NB - b0)
                st = spool.tile([P, SBK, P], BF16, name="st")
                nc.vector.tensor_tensor(
                    out=st[:, :k, :],
                    in0=iotab[:, 0:k * P].rearrange("p (k j) -> p k j", k=k),
                    in1=dstloc[:, b0:b0 + k].to_broadcast([P, k, P]),
                    op=mybir.AluOpType.is_equal)
                for j in range(k):
                    s_tiles.append((st, j))

            # ---- segment sum: per phase, 4-tile PSUM groups -> acc.
            # During the h1 phase each finished group immediately runs its
            # epilogue + next-layer GEMM + bounce write, so the next
            # AllGather launches while this layer's tail is still
            # aggregating (hides most of the collective latency).
            acc = accp.tile([P, NT, P], F32, name="acc")
            if layer < 2:
                x_next = sb.tile([P, NT, P], BF16, name="x_next")
                h_next = sb.tile([P, NT, P], BF16, name="h_sent")
                bounce = dram.tile([S, P], BF16, name="bounce")
            else:
                out_sb = sb.tile([P, NT, 64], F32, name="out_sb")
            next_tables = [None, None]
            gb = 0
            boff = 0
            for hh in range(2):
                for g in range(0, NT, TG):
                    gsz = min(TG, NT - g)
                    a_ps = ps_agg.tile([P, TG, P], F32, space="PSUM", name="a_ps")
                    have = []  # slices written this phase
                    for j in range(gsz):
                        t = g + j
                        nb = int(B[hh, t])
                        if hh == 0:
                            nc.tensor.matmul(
                                out=a_ps[:, j, :],
                                lhsT=sfake[:, t * P:(t + 1) * P],
                                rhs=bfake[:, layer, :], start=True, stop=False)
                            nc.tensor.matmul(
                                out=a_ps[:, j, :], lhsT=identb[:],
                                rhs=h_sent[:, t, :], start=False, stop=(nb == 0))
                        elif nb == 0:
                            continue
                        have.append(j)
                        for i in range(nb):
                            st, slot = s_tiles[gb]
                            nc.tensor.matmul(
                                out=a_ps[:, j, :], lhsT=st[:, slot, :],
                                rhs=msg_ap(boff),
                                start=(hh == 1 and i == 0), stop=(i == nb - 1))
                            gb += 1
                            boff += P
                    if hh == 0:
                        nc.vector.tensor_copy(acc[:, g:g + gsz, :], a_ps[:, :gsz, :])
                        continue
                    if len(have) == gsz:
                        nc.vector.tensor_tensor(
                            out=acc[:, g:g + gsz, :], in0=acc[:, g:g + gsz, :],
                            in1=a_ps[:, :gsz, :], op=mybir.AluOpType.add)
                    else:
                        for j in have:
                            nc.vector.tensor_tensor(
                                out=acc[:, g + j, :], in0=acc[:, g + j, :],
                                in1=a_ps[:, j, :], op=mybir.AluOpType.add)
                    # ---- per-group tail (h1 phase) ----
                    if layer < 2:
                        for t in range(g, g + gsz):
                            nc.scalar.activation(
                                out=x_next[:, t, :], in_=acc[:, t, :],
                                func=mybir.ActivationFunctionType.Relu,
                                scale=dinv2c[:, t:t + 1])
                        emit_gemm_group(layer + 1, g, gsz,
                                        lambda t: x_next[:, t, :], h_next)
                        dma_rows(bounce, h_next, g * P,
                                 min((g + gsz) * P, S))
                        if next_tables[0] is None and (g + gsz) * P >= HS:
                            next_tables[0] = emit_ag(bounce, 0, "tbl0")
                    else:
                        for t in range(g, g + gsz):
                            nc.scalar.activation(
                                out=out_sb[:, t, :], in_=acc[:, t, :64],
                                func=mybir.ActivationFunctionType.Copy,
                                scale=dinvc[:, t:t + 1])
                        r0, r1 = g * P, min((g + gsz) * P, n_full * P)
                        if r1 > r0:
                            nc.sync.dma_start(
                                out_ext[r0:r1, :].rearrange(
                                    "(t p) f -> p t f", p=P),
                                out_sb[:, g:g + (r1 - r0) // P, :])
                        if rem and g + gsz == NT:
                            nc.sync.dma_start(
                                out_ext[n_full * P:S, :].rearrange(
                                    "(t p) f -> p t f", t=1),
                                out_sb[0:rem, n_full:NT, :])
            if layer < 2:
                next_tables[1] = emit_ag(bounce, 1, "tbl1")
                tables = next_tables
                h_sent = h_next

    nc.compile()
    return nc


def make_in_maps(x, W1, b1, W2, b2, W3, b3, sched, per_core, dinv):
    """Build per-core input dicts (x pre-scaled by dinv)."""
    S, NT = sched["S"], sched["NT"]
    n_cores = sched["n_cores"]
    bf = ml_dtypes.bfloat16
    w1 = np.asarray(W1, np.float32).astype(bf)
    w2 = np.asarray(W2, np.float32).astype(bf)
    w3 = np.zeros((P, P), np.float32)
    w3[:, :64] = np.asarray(W3, np.float32)
    w3 = w3.astype(bf)
    bfake = np.zeros((P, 3, P), np.float32)
    bfake[0, 0, :] = np.asarray(b1, np.float32)
    bfake[0, 1, :] = np.asarray(b2, np.float32)
    bfake[0, 2, :64] = np.asarray(b3, np.float32)
    bfake = bfake.astype(bf)
    identb = np.eye(P, dtype=np.float32).astype(bf)
    iotab = np.tile(np.arange(P, dtype=np.float32), (P, SBK)).astype(bf)
    xs = np.asarray(x, np.float32) * np.asarray(dinv)[:, None]

    in_maps = []
    for c in range(n_cores):
        d = per_core[c]
        xp = np.zeros((NT * P, P), np.float32)
        xp[:S] = xs[c * S:(c + 1) * S]
        in_maps.append({
            "x": xp,
            "w1": w1, "w2": w2, "w3": w3,
            "bfake": bfake,
            "sfake": np.ascontiguousarray(d["sfake"]),
            "dinvc": np.ascontiguousarray(d["dinvc"]),
            "dinv2c": np.ascontiguousarray(d["dinv2c"]),
            "identb": identb, "iotab": iotab,
            "idxw": np.ascontiguousarray(d["idxw"]),
            "dstloc": np.ascontiguousarray(d["dstloc"]),
        })
    return in_maps


# ---------------------------------------------------------------------------
# Entry point: full inputs in, full output out.  Hardcoded problem shapes.
# ---------------------------------------------------------------------------
N_NODES = 50000
N_CORES = 8
CALL_SIZE = 1024


def _run(inputs, trace=False):
    from concourse.bass_utils import run_bass_kernel_spmd

    x = np.asarray(inputs["x"], np.float32)
    edge_index = np.asarray(inputs["edge_index"])
    sched, per_core, dinv = preprocess(edge_index, N_NODES, N_CORES, CALL_SIZE)
    nc = build_nc(sched)
    in_maps = make_in_maps(x, inputs["W1"], inputs["b1"], inputs["W2"],
                           inputs["b2"], inputs["W3"], inputs["b3"],
                           sched, per_core, dinv)
    res = run_bass_kernel_spmd(nc, in_maps, list(range(N_CORES)), trace=trace)
    out = np.concatenate([np.asarray(res.results[c]["out"])
                          for c in range(N_CORES)], axis=0)
    return out.astype(np.float32), res


def kernel(x, edge_index, W1, b1, W2, b2, W3, b3):
    out, _ = _run(dict(x=x, edge_index=edge_index, W1=W1, b1=b1, W2=W2,
                       b2=b2, W3=W3, b3=b3), trace=False)
    return out

